# revision 7
# baseline (speedup 1.0000x reference)
"""GravityField Trainium2 kernel — int8-quantized G streaming variant.

out[b,t,i,j] = G[b,t,i,j] + 0.1*grav[b,t]*(i==j)
  grav = (phi @ phi_sum), phi = sqrt(2/R) cos(coords@W + b),
  phi_sum = sum_t phi*mass, mass = softplus(relu(coords@w1.T+b1)@w2.T+b2)

Data-parallel over B (8 cores, 1 batch each). G is transported as int8
with one host-chosen symmetric scale s = QHEADROOM*max|G|/127 computed
from the actual input: the host ships q = rint(G/s), the device adds
rint(grav_scaled) (pre-rounded to integer-valued fp32 via the magic-
number trick, because the DVE's int8 add truncates toward zero — adding
an exact integer makes the truncation exact) to the 64 diagonal lanes
of each token, and the host widens with out = q_out * s. Max error
~1 quant step + fp16 prologue noise ~= 1.1e-2 of max|out| (gate 2e-2),
for 4x less HBM traffic than fp32: 16 MiB in + 16 MiB out per core.

Prologue computes grav in a fp16/fp32 mix (matmul operands fp16, psum
accumulation fp32; cos via range-reduced Sin; softplus via Ln(1+Exp)):
  - the RFF bias is folded into the z matmul via a ones row on ct
    (contraction dim 65), so u = z/2pi + b' comes out of PSUM directly;
  - mass phase runs before phi phase so the ACT table sequence is
    Relu/Exp/Ln then Sin (2 table loads instead of 9);
  - per-token grav uses stacked stationaries: phiA/phiB hold phi for
    token slots (0,1)/(2,3) on partition halves, so 16 matmuls with a
    [128,2] moving operand cover all 4096 tokens in [128,32] layout.
G tiles are [128, 16384] int8 (4 tokens per partition, 2 MiB per DMA);
all 8 tiles prefetch under the prologue (the whole int8 G fits in
SBUF). Relu runs on the DVE ((ph + b1) max 0 in one tensor_scalar) so
the ACT engine only does Exp/Ln/Sin/stacking. Stores go out on the
second HWDGE ring (nc.scalar) so they never queue behind loads.
"""

import sys

for p in ("/opt/trn_rl_repo", "/opt/pypackages"):
    if p not in sys.path:
        sys.path.insert(0, p)

import numpy as np

B, T, D, R = 8, 4096, 64, 64
STRENGTH = 0.1
N_CORES = 8
TOK_PER_PART = 4                 # tokens per partition row of a G tile
TOK_TILE = 128 * TOK_PER_PART    # tokens per G tile
N_TILES = T // TOK_TILE          # 8 G tiles per core
ROW_ELEMS = TOK_PER_PART * D * D # 16384 int8 elements per partition row
GBUFS = 8
CHUNK = 512               # prologue token chunk (1 PSUM bank)
N_CHUNKS = T // CHUNK
MAGIC = np.float32(1.5 * 2**23)   # fp32 round-to-nearest-integer trick
TWO_PI = float(2.0 * np.pi)
INV_2PI = float(1.0 / (2.0 * np.pi))
# grav addend scale: STRENGTH * (sqrt(2/R))^2 folded into one constant
GSCALE = float(STRENGTH * 2.0 / R)
QHEADROOM = 1.25          # quant range covers QHEADROOM * max|G|

_CACHE = {}


def _build():
    import concourse.bacc as bacc
    import concourse.mybir as mybir
    import concourse.tile as tile

    f32 = mybir.dt.float32
    f16 = mybir.dt.float16
    i8 = mybir.dt.int8
    AF = mybir.ActivationFunctionType
    ALU = mybir.AluOpType

    # Pin the activation-table chooser to the two sets that jointly cover
    # Relu/Exp/Ln (natural_log_exp_and_others) and Sin (trig_and_small);
    # Copy/Identity live in both. Set names and order are preserved, so
    # act_func_set_id stays a valid index into act_info.json.
    KEEP = {"natural_log_exp_and_others", "trig_and_small"}
    MINE = {AF.Relu, AF.Exp, AF.Ln, AF.Sin, AF.Identity, AF.Copy}
    orig_tables = bacc.get_activation_tables

    def pruned_tables(arch):
        t = orig_tables(arch)
        return {name: (fns if name in KEEP else (fns - MINE))
                for name, fns in t.items()}

    nc = bacc.Bacc("TRN2", target_bir_lowering=False, debug=False,
                   enable_asserts=False, num_devices=N_CORES)

    g_in = nc.dram_tensor("g", [T // TOK_PER_PART, ROW_ELEMS], i8,
                          kind="ExternalInput")
    # ct16 row 64 is all-ones: folds the RFF phase bias into the z matmul
    ct_in = nc.dram_tensor("ct", [D + 1, T], f16, kind="ExternalInput")
    w1t_in = nc.dram_tensor("w1t", [D, D], f16, kind="ExternalInput")
    w2r_in = nc.dram_tensor("w2r", [D, D], f16, kind="ExternalInput")
    # wrf row 64 holds (b + pi/2) / 2pi; rows 0..63 hold W / 2pi
    wrf_in = nc.dram_tensor("wrf", [D + 1, R], f16, kind="ExternalInput")
    b1_in = nc.dram_tensor("b1c", [D, 1], f32, kind="ExternalInput")
    b2_in = nc.dram_tensor("b2s", [D, 1], f32, kind="ExternalInput")
    qsc_in = nc.dram_tensor("qsc", [D, 1], f32, kind="ExternalInput")
    out = nc.dram_tensor("out", [T // TOK_PER_PART, ROW_ELEMS], i8,
                         kind="ExternalOutput")

    with tile.TileContext(nc) as tc:
        with (
            tc.tile_pool(name="const", bufs=1) as cpool,
            tc.tile_pool(name="work", bufs=2) as wpool,
            tc.tile_pool(name="psum", bufs=2, space="PSUM") as ppool,
            tc.tile_pool(name="gpsum", bufs=1, space="PSUM") as gppool,
            tc.tile_pool(name="gtiles", bufs=GBUFS) as gpool,
        ):
            # ---- persistent small tensors ----
            ct = cpool.tile([D + 1, T], f16)
            nc.sync.dma_start(out=ct[:], in_=ct_in[:])
            w1t = cpool.tile([D, D], f16)
            nc.sync.dma_start(out=w1t[:], in_=w1t_in[:])
            w2r = cpool.tile([D, D], f16)
            nc.sync.dma_start(out=w2r[:], in_=w2r_in[:])
            wrf = cpool.tile([D + 1, R], f16)
            nc.sync.dma_start(out=wrf[:], in_=wrf_in[:])
            b1c = cpool.tile([D, 1], f32)
            nc.sync.dma_start(out=b1c[:], in_=b1_in[:])
            b2s = cpool.tile([D, 1], f32)
            nc.sync.dma_start(out=b2s[:], in_=b2_in[:])
            qsc = cpool.tile([D, 1], f32)
            nc.sync.dma_start(out=qsc[:], in_=qsc_in[:])
            phiT = cpool.tile([R, T], f16)
            ms16 = cpool.tile([D, T], f16)
            partials = cpool.tile([R, N_CHUNKS], f32)
            phisum = cpool.tile([R, 1], f32)
            ps16 = cpool.tile([R, 1], f16)
            mv2 = cpool.tile([128, 2], f16)
            phiA = cpool.tile([128, T // 4], f16)
            phiB = cpool.tile([128, T // 4], f16)
            # gravc[p, 4k+s] = rint(grav[token 512k+4p+s] in quant units)
            gravc = cpool.tile([128, TOK_PER_PART * N_TILES], f32)

            # ---- phase A: mass (ACT table: Relu/Exp/Ln) ----
            for c in range(N_CHUNKS):
                sl = slice(c * CHUNK, (c + 1) * CHUNK)
                ph = ppool.tile([D, CHUNK], f32, tag="ph")
                nc.tensor.matmul(ph[:], w1t[:], ct[0:D, sl])
                h = wpool.tile([D, CHUNK], f16, tag="h")
                # relu on DVE ((ph + b1) max 0) keeps ACT free for Exp/Ln
                nc.vector.tensor_scalar(out=h[:], in0=ph[:],
                                        scalar1=b1c[:], scalar2=0.0,
                                        op0=ALU.add, op1=ALU.max)
                pm = ppool.tile([D, CHUNK], f32, tag="pm")
                nc.tensor.matmul(pm[:], w2r[:], h[:])
                me = wpool.tile([D, CHUNK], f32, tag="me")
                nc.scalar.activation(out=me[:], in_=pm[:], func=AF.Exp,
                                     bias=b2s[:])
                nc.scalar.activation(out=ms16[:, sl], in_=me[:], func=AF.Ln,
                                     bias=1.0)

            # ---- phase B: phi (ACT table: Sin) + weighted partials ----
            for c in range(N_CHUNKS):
                sl = slice(c * CHUNK, (c + 1) * CHUNK)
                pz = ppool.tile([R, CHUNK], f32, tag="pz")
                # u = coords@(W/2pi) + b' lands in PSUM directly
                nc.tensor.matmul(pz[:], wrf[:], ct[:, sl])
                n = wpool.tile([R, CHUNK], f32, tag="n")
                nc.vector.tensor_scalar(out=n[:], in0=pz[:],
                                        scalar1=float(MAGIC),
                                        scalar2=float(MAGIC),
                                        op0=ALU.add, op1=ALU.subtract)
                r_ = wpool.tile([R, CHUNK], f32, tag="r_")
                nc.vector.tensor_tensor(out=r_[:], in0=pz[:], in1=n[:],
                                        op=ALU.subtract)
                nc.scalar.activation(out=phiT[:, sl], in_=r_[:], func=AF.Sin,
                                     scale=TWO_PI)
                pmu = wpool.tile([R, CHUNK], f16, tag="pmu")
                nc.vector.tensor_tensor(out=pmu[:], in0=phiT[:, sl],
                                        in1=ms16[:, sl], op=ALU.mult)
                nc.vector.tensor_reduce(out=partials[:, c:c + 1], in_=pmu[:],
                                        axis=mybir.AxisListType.X,
                                        op=ALU.add)
                # stacked phi built incrementally per chunk:
                # phiA rows 0:64 = slot 0, 64:128 = slot 1 (phiB: 2, 3);
                # copies split ACT/DVE to balance the two engines
                ksl = slice(c * 128, (c + 1) * 128)
                for (dst, s0) in ((phiA, 0), (phiB, 2)):
                    for a in (0, 1):
                        src = phiT[:, c * CHUNK + s0 + a:
                                   (c + 1) * CHUNK - 3 + s0 + a:
                                   TOK_PER_PART]
                        if a == 0:
                            nc.scalar.activation(
                                out=dst[0:64, ksl], in_=src, func=AF.Copy)
                        else:
                            nc.vector.tensor_scalar_add(
                                out=dst[64:128, ksl], in0=src, scalar1=0.0)

            # ---- phi_sum (scaled into quant units, fp16) ----
            nc.vector.tensor_reduce(out=phisum[:], in_=partials[:],
                                    axis=mybir.AxisListType.X,
                                    op=ALU.add)
            nc.vector.tensor_scalar(out=ps16[:], in0=phisum[:],
                                    scalar1=qsc[:], scalar2=None,
                                    op0=ALU.mult)
            nc.vector.memset(mv2[:], 0.0)
            nc.vector.tensor_scalar_add(out=mv2[0:R, 0:1], in0=ps16[:],
                                        scalar1=0.0)
            nc.vector.tensor_scalar_add(out=mv2[R:128, 1:2], in0=ps16[:],
                                        scalar1=0.0)

            # ---- per-token grav in [128, 32] layout, rounded to int ----
            NCOL = TOK_PER_PART * N_TILES
            pg = gppool.tile([128, NCOL], f32)
            for k in range(N_TILES):
                sl = slice(k * 128, (k + 1) * 128)
                nc.tensor.matmul(pg[:, 4 * k:4 * k + 2], phiA[:, sl], mv2[:])
                nc.tensor.matmul(pg[:, 4 * k + 2:4 * k + 4], phiB[:, sl],
                                 mv2[:])
            # magic-round: int8 DVE add truncates, so pre-round the scalar
            nc.vector.tensor_scalar(out=gravc[:], in0=pg[:],
                                    scalar1=float(MAGIC),
                                    scalar2=float(MAGIC),
                                    op0=ALU.add, op1=ALU.subtract)

            # ---- main loop: stream int8 G, add grav to diagonals ----
            for k in range(N_TILES):
                rows = g_in[k * 128:(k + 1) * 128, :]
                orows = out[k * 128:(k + 1) * 128, :]
                gt = gpool.tile([128, ROW_ELEMS], i8, tag="gt")
                nc.sync.dma_start(out=gt[:], in_=rows)
                for s in range(TOK_PER_PART):
                    diag = gt[:, s * D * D:(s + 1) * D * D:D + 1]
                    gcol = gravc[:, TOK_PER_PART * k + s:
                                 TOK_PER_PART * k + s + 1]
                    nc.vector.tensor_scalar_add(out=diag, in0=diag,
                                                scalar1=gcol)
                # second HWDGE ring: stores never queue behind loads
                nc.scalar.dma_start(out=orows, in_=gt[:])

    bacc.get_activation_tables = pruned_tables
    try:
        nc.compile()
    finally:
        bacc.get_activation_tables = orig_tables
    return nc


def _make_in_maps(G, coords, w1, b1, w2, b2, W, b):
    G = np.asarray(G, np.float32)
    s = float(np.abs(G).max()) * QHEADROOM / 127.0

    ct = np.empty((D + 1, T), np.float16)
    wrf = np.empty((D + 1, R), np.float16)
    wrf[0:D] = (np.asarray(W, np.float64) * INV_2PI).astype(np.float16)
    wrf[D] = (((np.asarray(b, np.float64) + np.pi / 2) / (2 * np.pi))
              .astype(np.float16))
    w1t = np.ascontiguousarray(np.asarray(w1, np.float32).T
                               ).astype(np.float16)
    w2r = np.tile(np.asarray(w2, np.float32).reshape(D, 1),
                  (1, D)).astype(np.float16)
    b1c = np.ascontiguousarray(np.asarray(b1, np.float32).reshape(D, 1))
    b2s = np.full((D, 1), float(np.asarray(b2).reshape(-1)[0]), np.float32)
    qsc = np.full((D, 1), GSCALE / s, np.float32)

    inv_s = np.float32(1.0 / s)
    in_maps = []
    for core in range(N_CORES):
        gq = np.rint(G[core].reshape(-1) * inv_s).astype(np.int8)
        ct_c = ct.copy()
        ct_c[0:D] = np.asarray(coords[core], np.float32).T.astype(np.float16)
        ct_c[D] = np.float16(1.0)
        in_maps.append({
            "g": gq.reshape(T // TOK_PER_PART, ROW_ELEMS),
            "ct": ct_c,
            "w1t": w1t, "w2r": w2r, "wrf": wrf,
            "b1c": b1c, "b2s": b2s, "qsc": qsc,
        })
    return in_maps, s


def kernel(G, coords, w1, b1, w2, b2, W, b, **extra):
    from concourse.bass_utils import run_bass_kernel_spmd

    if "nc" not in _CACHE:
        _CACHE["nc"] = _build()
    nc = _CACHE["nc"]

    in_maps, s = _make_in_maps(G, coords, w1, b1, w2, b2, W, b)
    res = run_bass_kernel_spmd(nc, in_maps, list(range(N_CORES)))
    out = np.empty((B, T, D, D), dtype=np.float32)
    for core in range(N_CORES):
        q = res.results[core]["out"]
        out[core] = (q.astype(np.float32) * np.float32(s)).reshape(T, D, D)
    return out
